# revision 18
# baseline (speedup 1.0000x reference)
"""Trainium2 Bass kernel for nn_CausalSelfAttention_28467043237962.

Sharding: 8 cores = 2 batches x 4 head-groups (4 heads / 256 dims each).

v2 design (vs the 235us baseline):
- t-tile-major QKV: one [128,1024]f32 PSUM slot per t-tile (two chunk writes),
  raw bf16 copy to SBUF, squares/reduce for RMS stats.
- rstd via exp(-0.5*ln(m)) on the scalar engine: Ln/Exp/Copy all live in one
  activation-table set, so the table loads once (no Sqrt thrashing).  The
  SCALE and lambda0 factors ride the exp bias (ln SCALE / ln lambda0).
- q/k transposes via the DMA XBAR (dma_start transpose=True), not the PE.
- Attention in transposed-scores layout (scores_T[s,t]); softmax denominators
  from ones-columns inside the AV matmul; per (h, jj-half) the i-loop is
  software-pipelined (sc_{i+1} emitted before av_i) so the Act exp latency
  hides behind PE work.
- PE fillers keep the tensor engine dense (HAM stays at K=8/8): QKV quarters
  2,3 are emitted inside attention block jj=0; the jj=0 output projection is
  emitted inside attention block jj=1.
- AV matmuls and exp are trimmed at the causal diagonal; the tri mask
  multiply only touches the 128-wide diagonal strip (gpsimd engine).
- PSUM: tag "sc" [128,1024]f32 x2 shared by scores / QKV / out-proj chunks,
  tag "av" [128,1024]f32 x2.  16KB exactly.

Self-contained: builds one SPMD Bass program and runs it on cores 0-7 via
concourse.bass_utils.run_bass_kernel_spmd.
"""
import sys

sys.path.insert(0, "/opt/trn_rl_repo")

from contextlib import ExitStack

import numpy as np
import ml_dtypes

import concourse.bass as bass
import concourse.tile as tile
import concourse.mybir as mybir
from concourse.vector_clock import ScopedClock
from concourse.bass_utils import run_bass_kernel_spmd

F32 = mybir.dt.float32
BF16 = mybir.dt.bfloat16

B, T, DIM = 2, 2048, 1024
H, HD = 16, 64
SCALE = 0.12
ROPE_BASE = 1024.0
EPS = 1e-6
G = 4          # head-groups = cores per batch
HPG = 4        # heads per group
GD = HPG * HD  # 256 dims per group
NT = T // 128  # 16 s/t tiles of 128
NJ = T // 512  # 4 t-blocks of 512

TRACE = False          # set by test.py for profiling runs
DBG = False
LAST_EXEC_NS = None    # filled when TRACE


class _TileContextFixed(tile.TileContext):
    """Workaround for this container's walrus build: the kernel-tail InstDrain
    may carry only one sync wait. Spread the tail waits over single-wait NOPs
    on the sync engine before a wait-free drain."""

    def _drain_and_barrier(self, tick_clock, wait_clock):
        nc = self.nc
        collector = nc.sync.nop(nofuse=True, hint="tail_wait_collector")
        wait_clock.add_sem_waits(
            collector.ins, ScopedClock({None: tick_clock.global_clock})
        )
        si = collector.ins.sync_info
        waits = list(si.on_wait or [])
        if len(waits) > 1:
            si.on_wait = waits[:1]
            for w in waits[1:]:
                extra = nc.sync.nop(nofuse=True, hint="tail_wait")
                esi = extra.ins.sync_info
                if esi is None:
                    extra.ins.sync_info = mybir.SyncInfo(on_wait=[w], on_update=[])
                else:
                    esi.on_wait = [w]
        nc.sync.drain()
        nc.all_engine_barrier()
        assert self.sems is not None
        popped = nc._tile_sem_poison_stack.pop()
        assert popped is self._sem_poison
        nc.clear_and_free_semaphores(list(self.sems.allocated().values()))
        nc.all_engine_barrier()


def _split_excess_waits(nc, max_waits=1):
    """This container's walrus build rejects instructions carrying more than
    one embedded sync wait. Move excess waits onto dedicated NOPs inserted
    just before the instruction on the same engine."""
    ctr = [0]
    for func in nc.m.functions:
        for block in func.blocks:
            out = []
            for inst in block.instructions:
                si = inst.sync_info
                waits = list(si.on_wait) if si and si.on_wait else []
                limit = 0 if isinstance(inst, mybir.InstDrain) else max_waits
                if len(waits) > limit:
                    keep = waits[:limit]
                    extra = waits[limit:]
                    for w in extra:
                        ctr[0] += 1
                        nop = mybir.InstNoOp(
                            name=f"waitnop-{ctr[0]}",
                            sync_info=mybir.SyncInfo(on_wait=[w], on_update=[]),
                            bass_nofuse=True,
                            engine=inst.engine,
                        )
                        out.append(nop)
                    si.on_wait = keep
                out.append(inst)
            block.instructions = out


def _rope_tables():
    keep = HD // 4  # 16 active frequencies; dims 16:32 of each half are identity
    active = (1.0 / ROPE_BASE) ** np.linspace(0.0, 1.0, keep, dtype=np.float32)
    th = np.arange(T, dtype=np.float32)[:, None] * active[None, :]
    return np.cos(th).astype(np.float32), np.sin(th).astype(np.float32)


def _classify_blocks(mask):
    """mask [T,T] additive, indexed (t, s). Block = (s-tile i of 128) x
    (t-block jb of 512). Returns cls[i][jb] in {skip, pass, tri, gen}."""
    cls = []
    for i in range(NT):
        row = []
        for jb in range(NJ):
            blk = mask[512 * jb:512 * (jb + 1), 128 * i:128 * (i + 1)]
            big_neg = blk <= -1e8
            zero = blk == 0.0
            if big_neg.all():
                row.append("skip")
            elif zero.all():
                row.append("pass")
            elif (big_neg | zero).all():
                tt = np.arange(512 * jb, 512 * (jb + 1))[:, None]
                ss = np.arange(128 * i, 128 * (i + 1))[None, :]
                row.append("tri" if np.array_equal(zero, tt >= ss) else "gen")
            else:
                row.append("gen")
        cls.append(row)
    return cls


def _build_program(cls, has_gen):
    nc = bass.Bass()
    xP = nc.declare_dram_parameter("xP", [128, NT, 8, 128], BF16, isOutput=False)
    wqP = nc.declare_dram_parameter("wqP", [128, 8, 3 * GD], BF16, isOutput=False)
    woP = nc.declare_dram_parameter("woP", [128, 2, DIM], BF16, isOutput=False)
    veP = nc.declare_dram_parameter("veP", [128, NT, GD], BF16, isOutput=False)
    lamP = nc.declare_dram_parameter("lamP", [128, 1], F32, isOutput=False)
    rcP = nc.declare_dram_parameter("rcP", [128, NT, 16], BF16, isOutput=False)
    rsP = nc.declare_dram_parameter("rsP", [128, NT, 16], BF16, isOutput=False)
    triP = nc.declare_dram_parameter("triP", [128, 128], BF16, isOutput=False)
    md = None
    if has_gen:
        md = nc.declare_dram_parameter("maskdiv", [T, T], F32, isOutput=False)
    outP = nc.declare_dram_parameter("outP", [DIM, T], BF16, isOutput=True)
    rec_dram = nc.dram_tensor("rec_scratch", [HPG, T], BF16)

    # per-jb: first/last valid s-tile for AV accumulation start/stop
    first_i = [None] * NJ
    last_i = [None] * NJ
    for jb in range(NJ):
        valid = [i for i in range(NT) if cls[i][jb] != "skip"]
        if valid:
            first_i[jb] = valid[0]
            last_i[jb] = valid[-1]

    def block_trim(i, jb):
        if cls[i][jb] != "tri":
            return 0
        tr = 128 * (i - 4 * jb)
        # the first accumulating matmul must initialize the full 512 region
        if i == first_i[jb]:
            return 0
        return tr

    with _TileContextFixed(nc) as tc, ExitStack() as ctx:
        S = ctx.enter_context(tc.tile_pool(name="singles", bufs=1))

        # ---- SBUF singles
        x_sb = S.tile([128, NT, 8, 128], BF16, tag="x_sb")
        wq_sb = S.tile([128, 8, 3 * GD], BF16, tag="wq_sb")
        wo_sb = S.tile([128, 2, DIM], BF16, tag="wo_sb")
        ve_sb = S.tile([128, NT, GD], BF16, tag="ve_sb")
        lam_sb = S.tile([128, 1], F32, tag="lam_sb")
        rc_sb = S.tile([128, NT, 16], BF16, tag="rc_sb")
        rs_sb = S.tile([128, NT, 16], BF16, tag="rs_sb")
        trikeep_sb = S.tile([128, 128], BF16, tag="trikeep_sb")

        ident = S.tile([128, 128], BF16, tag="ident")
        from concourse.masks import make_identity
        make_identity(nc, ident)

        qkv_sb = S.tile([128, NT, 3 * GD], BF16, tag="qkv_sb")
        qkT = S.tile([128, 4, T], BF16, tag="qkT")
        yT = S.tile([128, 2, T], BF16, tag="yT")
        scl_sb = S.tile([128, NT, HPG], F32, tag="scl_sb")
        rstdq = S.tile([128, NT, 12], F32, tag="rstdq")
        rstdv = S.tile([128, NT, HPG], F32, tag="rstdv")
        ms = S.tile([128, NT, 12], F32, tag="ms")

        # ---- input DMAs.  queue SP: x tiles (prefetch-ordered; more emitted
        # inside emit_tile).  queue Act: weights + small tables.
        for ds in range(8):
            nc.scalar.dma_start(out=wq_sb[:, ds, :], in_=wqP[:, ds, :])
        x_fetched = [False] * NT

        def fetch_x(tt):
            if not x_fetched[tt]:
                x_fetched[tt] = True
                nc.sync.dma_start(out=x_sb[:, tt], in_=xP[:, tt])

        for tt in range(6):
            fetch_x(tt)
        nc.sync.dma_start(out=lam_sb, in_=lamP[:, :])
        nc.sync.dma_start(out=rc_sb, in_=rcP[:, :, :])
        nc.sync.dma_start(out=rs_sb, in_=rsP[:, :, :])
        nc.sync.dma_start(out=trikeep_sb, in_=triP[:, :])
        nc.scalar.dma_start(out=ve_sb, in_=veP[:, :, :])
        nc.scalar.dma_start(out=wo_sb, in_=woP[:, :, :])

        # v_aug[p, tt, h, 0:128]: even h -> [v | ones], odd h -> [ones | v]
        v_aug = S.tile([128, NT, HPG, 128], BF16, tag="v_aug")
        v5 = v_aug.rearrange("p t (a b) c -> p t a b c", b=2)
        nc.gpsimd.memset(v5[:, :, :, 0, 64:128], 1.0)
        nc.gpsimd.memset(v5[:, :, :, 1, 0:64], 1.0)

        # ---- pools
        PS = ctx.enter_context(tc.tile_pool(name="ps", bufs=2, space="PSUM"))
        EX = ctx.enter_context(tc.tile_pool(name="ex_sb", bufs=20))
        A = ctx.enter_context(tc.tile_pool(name="a_sb", bufs=2))
        NR = ctx.enter_context(tc.tile_pool(name="rec_sb", bufs=2))
        OS = ctx.enter_context(tc.tile_pool(name="os_sb", bufs=3))
        MD = ctx.enter_context(tc.tile_pool(name="md_sb", bufs=2))

        # ================= stage A =================
        sqs = {}
        pending_back = []
        pending_tp = []

        def emit_tile_front(tt):
            """QKV matmuls + raw copy + square for t-tile tt."""
            fetch_x(min(tt + 6, NT - 1))
            qp = PS.tile([128, 1024], F32, tag="sc", name="qp")
            for ds in range(8):
                nc.tensor.matmul(
                    qp[:, 0:512], x_sb[:, tt, ds, :], wq_sb[:, ds, 0:512],
                    start=(ds == 0), stop=(ds == 7))
            for ds in range(8):
                nc.tensor.matmul(
                    qp[:, 512:768], x_sb[:, tt, ds, :], wq_sb[:, ds, 512:768],
                    start=(ds == 0), stop=(ds == 7))
            nc.scalar.activation(
                qkv_sb[:, tt, :], qp[:, 0:768],
                mybir.ActivationFunctionType.Copy)
            sq = A.tile([128, 768], BF16, tag="sq", name="sq")
            sqs[tt] = sq
            nc.gpsimd.tensor_tensor(
                sq, qkv_sb[:, tt, :], qkv_sb[:, tt, :], mybir.AluOpType.mult)

        def emit_tile_back(tt):
            """stat chain + norm + rope + v-blend for t-tile tt (runs one
            filler period after the front so no engine head-of-line blocks)."""
            sq = sqs.pop(tt)
            nc.vector.tensor_reduce(
                ms[:, tt, :],
                sq.rearrange("p (g d) -> p g d", d=HD),
                axis=mybir.AxisListType.X, op=mybir.AluOpType.add)
            hsl = slice(tt, tt + 1)
            mm = A.tile([128, 1, 12], F32, tag="mm", name="mm")
            nc.vector.tensor_scalar(
                out=mm, in0=ms[:, hsl, :], scalar1=1.0 / HD, scalar2=EPS,
                op0=mybir.AluOpType.mult, op1=mybir.AluOpType.add)
            lnm = A.tile([128, 1, 12], F32, tag="lnm", name="lnm")
            nc.scalar.activation(
                lnm, mm, mybir.ActivationFunctionType.Ln)
            nc.scalar.activation(
                rstdq[:, hsl, :], lnm, mybir.ActivationFunctionType.Exp,
                scale=-0.5)
            nc.vector.tensor_scalar_mul(
                scl_sb[:, hsl, :], rstdq[:, hsl, 4:8], SCALE)
            nc.vector.tensor_scalar_mul(
                rstdv[:, hsl, :], rstdq[:, hsl, 8:12], lam_sb[:, 0:1])

            qk4 = qkv_sb[:, hsl, :].rearrange("p t (g d) -> p t g d", d=HD)
            if has_gen:
                nc.vector.tensor_tensor(
                    qk4[:, :, 4:8, :], qk4[:, :, 4:8, :],
                    rstdq[:, hsl, 4:8, None].to_broadcast([128, 1, 4, HD]),
                    mybir.AluOpType.mult)
            nc.vector.tensor_tensor(
                qk4[:, :, 0:4, :], qk4[:, :, 0:4, :],
                rstdq[:, hsl, 0:4, None].to_broadcast([128, 1, 4, HD]),
                mybir.AluOpType.mult)

            vn = A.tile([128, 1, 4, HD], BF16, tag="vn", name="vn")
            nc.vector.tensor_tensor(
                vn, qk4[:, :, 8:12, :],
                rstdv[:, hsl, :, None].to_broadcast([128, 1, 4, HD]),
                mybir.AluOpType.mult)
            vn4 = vn.rearrange("p t (a b) d -> p t a b d", b=2)
            vev = ve_sb[:, hsl, :].rearrange(
                "p t (a b d) -> p t a b d", a=2, d=HD)
            for a in range(2):
                nc.gpsimd.tensor_tensor(
                    v5[:, hsl, a, 0, 0:64], vn4[:, :, a, 0, :],
                    vev[:, :, a, 0, :], mybir.AluOpType.add)
                nc.gpsimd.tensor_tensor(
                    v5[:, hsl, a, 1, 64:128], vn4[:, :, a, 1, :],
                    vev[:, :, a, 1, :], mybir.AluOpType.add)

            v6 = qkv_sb[:, hsl, 0:512].rearrange(
                "p t (sg d) -> p t sg d", d=HD)
            x0 = v6[:, :, :, 0:16]
            x32 = v6[:, :, :, 32:48]
            cb = rc_sb[:, hsl, None, :].to_broadcast([128, 1, 8, 16])
            sb = rs_sb[:, hsl, None, :].to_broadcast([128, 1, 8, 16])
            ra = A.tile([128, 1, 8, 16], BF16, tag="ra", name="ra")
            rb = A.tile([128, 1, 8, 16], BF16, tag="rb", name="rb")
            nc.gpsimd.tensor_tensor(ra, x0, sb, mybir.AluOpType.mult)
            nc.gpsimd.tensor_tensor(rb, x32, sb, mybir.AluOpType.mult)
            nc.vector.tensor_tensor(x0, x0, cb, mybir.AluOpType.mult)
            nc.vector.tensor_tensor(x32, x32, cb, mybir.AluOpType.mult)
            nc.vector.tensor_tensor(x0, x0, rb, mybir.AluOpType.add)
            nc.vector.tensor_tensor(x32, x32, ra, mybir.AluOpType.subtract)

        def emit_transposes(tt):
            if tt < 8:
                # pure-a phase: PE identity-matmul transposes (sync queue is
                # busy streaming x then)
                ts = slice(128 * tt, 128 * (tt + 1))
                tp_t = PS.tile([128, 2048], BF16, tag="sc", name="tp_t")
                tp = tp_t[:, 0:512]
                for ec in range(4):
                    nc.tensor.transpose(
                        tp[:, 128 * ec:128 * (ec + 1)],
                        qkv_sb[:, tt, 128 * ec:128 * (ec + 1)], ident)
                nc.vector.tensor_scalar_mul(
                    qkT[:, :, ts],
                    tp.rearrange("p (e c) -> p e c", c=128), 1.0)
            else:
                # attention phase: DMA XBAR transposes (sync queue is free,
                # the PE is not)
                for ec in range(4):
                    nc.sync.dma_start(
                        out=qkT[:, ec, 128 * tt:128 * (tt + 1)],
                        in_=qkv_sb[:, tt, 128 * ec:128 * (ec + 1)],
                        transpose=True)

        def emit_tile(tt):
            emit_tile_front(tt)
            pending_back.append(tt)
            if len(pending_back) > 1:
                emit_tile_back(pending_back.pop(0))
            pending_tp.append(tt)
            if len(pending_tp) > 2:
                emit_transposes(pending_tp.pop(0))

        def flush_transposes():
            while pending_back:
                emit_tile_back(pending_back.pop(0))
            while pending_tp:
                emit_transposes(pending_tp.pop(0))

        # ================= stage B =================
        # Block-level software pipeline: while the PE runs the AV matmuls of
        # block X-1 (whose exps finished a block ago, buffered in SBUF ex
        # tiles), it interleaves the scores matmuls of block X and the Act
        # engine streams block X's exps.  The exp latency is thus never on
        # the PE's critical path, and the PE stays dense (HAM stays warm).
        exs = {}
        avs = {}

        def ivals_of(h, jj):
            jbsel = (2 * jj, 2 * jj + 1)
            return [i for i in range(NT)
                    if any(cls[i][jb] != "skip" for jb in jbsel)]

        def jbs_of(i, jj):
            return [jb for jb in (2 * jj, 2 * jj + 1) if cls[i][jb] != "skip"]

        def emit_sc_iter(h, jj, i):
            par, pair = h % 2, h // 2
            rlo, rhi = 64 * par, 64 * par + 64
            sc = PS.tile([128, 1024], F32, tag="sc", name="sc")
            jbs = jbs_of(i, jj)
            for jb in jbs:
                off = 512 * (jb - 2 * jj)
                trim = block_trim(i, jb)
                nc.tensor.matmul(
                    sc[:, off + trim:off + 512],
                    qkT[rlo:rhi, 2 + pair, 128 * i:128 * (i + 1)],
                    qkT[rlo:rhi, pair, 512 * jb + trim:512 * (jb + 1)],
                    start=True, stop=True)
                if cls[i][jb] == "gen":
                    mdt = MD.tile([128, 512], F32, tag="mdt", name="mdt")
                    nc.sync.dma_start(
                        out=mdt,
                        in_=md[512 * jb:512 * (jb + 1),
                               128 * i:128 * (i + 1)].rearrange("t s -> s t"))
                    nc.vector.tensor_tensor(
                        sc[:, off:off + 512], sc[:, off:off + 512],
                        mdt, mybir.AluOpType.add)
            ex = EX.tile([128, 1024], BF16, tag="ex", name="ex")
            exs[(h, jj, i)] = ex
            lo = 512 * (min(jbs) - 2 * jj) + block_trim(i, min(jbs))
            hi = 512 * (max(jbs) - 2 * jj) + 512
            escale = (SCALE if has_gen else scl_sb[:, i, h:h + 1])
            nc.scalar.activation(
                ex[:, lo:hi], sc[:, lo:hi],
                mybir.ActivationFunctionType.Exp, scale=escale)
            for jb in jbs:
                if cls[i][jb] != "tri":
                    continue
                # zero the above-diagonal half of the 128-wide strip
                # (gpsimd; consumed by the AV matmuls a block later)
                off = 512 * (jb - 2 * jj)
                strip = off + 128 * (i - 4 * jb)
                nc.gpsimd.tensor_tensor(
                    ex[:, strip:strip + 128],
                    ex[:, strip:strip + 128],
                    trikeep_sb, mybir.AluOpType.mult)

        def emit_av_iter(h, jj, jb, i):
            if (h, jj) not in avs:
                avs[(h, jj)] = PS.tile([128, 1024], F32, tag="av", name="av")
            av = avs[(h, jj)]
            ex = exs[(h, jj, i)]
            off = 512 * (jb - 2 * jj)
            trim = block_trim(i, jb)
            nc.tensor.matmul(
                av[:, off + trim:off + 512],
                v_aug[:, i, h, :],
                ex[:, off + trim:off + 512],
                start=(first_i[jb] == i),
                stop=(last_i[jb] == i))

        def emit_block_tail(h, jj):
            """den extraction + reciprocal + broadcast + y normalize."""
            par, pair = h % 2, h // 2
            rlo, rhi = 64 * par, 64 * par + 64
            dlo = 64 - rlo
            av = avs.pop((h, jj))
            for i in ivals_of(h, jj):
                exs.pop((h, jj, i), None)
            for jb in (2 * jj, 2 * jj + 1):
                if first_i[jb] is None:
                    off = 512 * (jb - 2 * jj)
                    nc.vector.memset(av[:, off:off + 512], 1.0)
            den_sb = NR.tile([1, 1024], F32, tag="den_sb", name="den_sb")
            nc.vector.tensor_scalar_mul(den_sb, av[dlo:dlo + 1, :], 1.0)
            den_pk = NR.tile([128, 8], F32, tag="den_pk", name="den_pk")
            nc.scalar.dma_start(out=den_pk, in_=den_sb)
            rec_pk = NR.tile([128, 8], F32, tag="rec_pk", name="rec_pk")
            nc.vector.reciprocal(rec_pk, den_pk)
            rec_bf = NR.tile([128, 8], BF16, tag="rec_bf", name="rec_bf")
            nc.vector.tensor_copy(out=rec_bf, in_=rec_pk)
            hsl2 = slice(1024 * jj, 1024 * (jj + 1))
            nc.scalar.dma_start(out=rec_dram[h, hsl2], in_=rec_bf)
            rrow_ap = rec_dram[h, hsl2]
            rec_bc = bass.AP(
                tensor=rrow_ap.tensor,
                offset=rrow_ap.offset,
                ap=[[0, 64]] + [list(p) for p in rrow_ap.ap])
            rec = NR.tile([64, 1024], BF16, tag="rec", name="rec")
            nc.scalar.dma_start(out=rec, in_=rec_bc)
            nc.vector.tensor_tensor(
                yT[rlo:rhi, pair, hsl2],
                av[rlo:rhi, :], rec,
                mybir.AluOpType.mult)

        def run_block_pair(blk, prev, fillers, cadence):
            """Interleave scores/exp of `blk` with AVs of `prev`."""
            sc_ops = ([lambda h=blk[0], jj=blk[1], i=i: emit_sc_iter(h, jj, i)
                       for i in ivals_of(*blk)] if blk else [])
            av_ops = []
            if prev:
                hP, jjP = prev
                for jb in (2 * jjP, 2 * jjP + 1):
                    for i in range(NT):
                        if cls[i][jb] != "skip":
                            av_ops.append(
                                lambda h=hP, jj=jjP, jb=jb, i=i:
                                emit_av_iter(h, jj, jb, i))
            seq = ([(((k + 0.5) / len(sc_ops)), 0, f)
                    for k, f in enumerate(sc_ops)]
                   + [(((k + 0.5) / len(av_ops)), 1, f)
                      for k, f in enumerate(av_ops)])
            seq.sort(key=lambda t: (t[0], t[1]))
            for n, (_, _, f) in enumerate(seq):
                f()
                if fillers and n % cadence == cadence - 1:
                    fillers.pop(0)()
            if prev:
                emit_block_tail(*prev)

        # ================= stage C =================
        def emit_po(ec, tb):
            po = PS.tile([128, 1024], F32, tag="sc", name="po")
            for dc in range(2):
                nc.tensor.matmul(
                    po[:, 0:512],
                    wo_sb[:, dc, 128 * ec:128 * (ec + 1)],
                    yT[:, dc, 512 * tb:512 * (tb + 1)],
                    start=(dc == 0), stop=(dc == 1))
            os = OS.tile([128, 512], BF16, tag="os", name="os")
            nc.vector.tensor_copy(out=os, in_=po[:, 0:512])
            nc.sync.dma_start(
                out=outP[128 * ec:128 * (ec + 1), 512 * tb:512 * (tb + 1)],
                in_=os)

        # ================= emission =================
        for tt in range(10):
            emit_tile(tt)
        flush_transposes()

        fillers = [lambda tt=tt: emit_tile(tt) for tt in range(10, 16)]
        fillers.append(flush_transposes)
        blocks = [(h, 0) for h in range(HPG)] + [(h, 1) for h in range(HPG)]
        po_started = [False]

        def add_po_fillers():
            if not po_started[0]:
                po_started[0] = True
                for ec in range(8):
                    for tb in (0, 1):
                        fillers.append(
                            lambda ec=ec, tb=tb: emit_po(ec, tb))

        prev = None
        for bi, blk in enumerate(blocks):
            if bi == HPG:
                flush_transposes()
            if bi == HPG + 1:
                # all jj0 tails are emitted once prev==(h3,0) is retired
                add_po_fillers()
            run_block_pair(blk, prev, fillers, 3 if bi < 2 else 5)
            prev = blk
        add_po_fillers()
        run_block_pair(None, prev, fillers, 3)

        while fillers:
            fillers.pop(0)()
        for ec in range(8):
            for tb in (2, 3):
                emit_po(ec, tb)
    _split_excess_waits(nc)
    return nc


def kernel(x, ve, sa_lambdas, attn_mask, qkvo_w):
    global LAST_EXEC_NS
    x = np.ascontiguousarray(np.asarray(x, np.float32))
    ve = np.ascontiguousarray(np.asarray(ve, np.float32))
    sa_lambdas = np.asarray(sa_lambdas, np.float32)
    attn_mask = np.asarray(attn_mask, np.float32)
    qkvo_w = np.asarray(qkvo_w, np.float32)

    ropeC, ropeS = _rope_tables()
    mask = attn_mask[0, 0]
    cls = _classify_blocks(mask)
    for jb in range(NJ):
        valid = [i for i in range(NT) if cls[i][jb] != "skip"]
        if valid and cls[valid[0]][jb] == "tri" and valid[0] - 4 * jb > 0:
            # the tri fast path assumes the strip starts inside the block
            cls[valid[0]][jb] = "gen"
    has_gen = any(c == "gen" for row in cls for c in row)

    nc = _build_program(cls, has_gen)

    part = np.arange(128)
    trikeep = (part[:, None] <= part[None, :]).astype(np.float32)      # [p, c]
    trikeep = np.ascontiguousarray(trikeep).astype(ml_dtypes.bfloat16)
    lam0 = np.full((128, 1), sa_lambdas[0], np.float32)
    rcP = np.ascontiguousarray(
        ropeC.reshape(NT, 128, 16).transpose(1, 0, 2)).astype(ml_dtypes.bfloat16)
    rsP = np.ascontiguousarray(
        ropeS.reshape(NT, 128, 16).transpose(1, 0, 2)).astype(ml_dtypes.bfloat16)
    maskdiv = (mask / SCALE).astype(np.float32) if has_gen else None

    in_maps = []
    for c in range(8):
        b, g = c // G, c % G
        sl = slice(GD * g, GD * (g + 1))
        wqkvT = np.concatenate([qkvo_w[k][sl, :] for k in range(3)], 0).T
        # xP[p, tt, ds, c] = x[b][128*tt + c, 128*ds + p]
        xco = np.ascontiguousarray(
            x[b].T.reshape(8, 128, NT, 128).transpose(1, 2, 0, 3)
        ).astype(ml_dtypes.bfloat16)
        m = {
            "xP": xco,
            "wqP": np.ascontiguousarray(
                wqkvT.reshape(8, 128, 3 * GD).transpose(1, 0, 2)
            ).astype(ml_dtypes.bfloat16),
            "woP": np.ascontiguousarray(
                qkvo_w[3][:, sl].T.reshape(2, 128, DIM).transpose(1, 0, 2)
            ).astype(ml_dtypes.bfloat16),
            "veP": np.ascontiguousarray(
                (ve[b][:, sl] * sa_lambdas[1]).reshape(NT, 128, GD)
                .transpose(1, 0, 2)
            ).astype(ml_dtypes.bfloat16),
            "lamP": lam0,
            "rcP": rcP,
            "rsP": rsP,
            "triP": trikeep,
        }
        if has_gen:
            m["maskdiv"] = maskdiv
        in_maps.append(m)

    res = run_bass_kernel_spmd(nc, in_maps, core_ids=list(range(8)),
                               trace=TRACE)
    if TRACE:
        LAST_EXEC_NS = res.exec_time_ns

    out = np.zeros((B, T, DIM), np.float32)
    for c in range(8):
        out[c // G] += res.results[c]["outP"].astype(np.float32).T
    return out


# revision 19
# speedup vs baseline: 1.0434x; 1.0434x over previous
"""Trainium2 Bass kernel for nn_CausalSelfAttention_28467043237962.

Sharding: 8 cores = 2 batches x 4 head-groups (4 heads / 256 dims each).

v2 design (vs the 235us baseline):
- t-tile-major QKV: one [128,1024]f32 PSUM slot per t-tile (two chunk writes),
  raw bf16 copy to SBUF, squares/reduce for RMS stats.
- rstd via exp(-0.5*ln(m)) on the scalar engine: Ln/Exp/Copy all live in one
  activation-table set, so the table loads once (no Sqrt thrashing).  The
  SCALE and lambda0 factors ride the exp bias (ln SCALE / ln lambda0).
- q/k transposes via the DMA XBAR (dma_start transpose=True), not the PE.
- Attention in transposed-scores layout (scores_T[s,t]); softmax denominators
  from ones-columns inside the AV matmul; per (h, jj-half) the i-loop is
  software-pipelined (sc_{i+1} emitted before av_i) so the Act exp latency
  hides behind PE work.
- PE fillers keep the tensor engine dense (HAM stays at K=8/8): QKV quarters
  2,3 are emitted inside attention block jj=0; the jj=0 output projection is
  emitted inside attention block jj=1.
- AV matmuls and exp are trimmed at the causal diagonal; the tri mask
  multiply only touches the 128-wide diagonal strip (gpsimd engine).
- PSUM: tag "sc" [128,1024]f32 x2 shared by scores / QKV / out-proj chunks,
  tag "av" [128,1024]f32 x2.  16KB exactly.

Self-contained: builds one SPMD Bass program and runs it on cores 0-7 via
concourse.bass_utils.run_bass_kernel_spmd.
"""
import sys

sys.path.insert(0, "/opt/trn_rl_repo")

from contextlib import ExitStack

import numpy as np
import ml_dtypes

import concourse.bass as bass
import concourse.tile as tile
import concourse.mybir as mybir
from concourse.vector_clock import ScopedClock
from concourse.bass_utils import run_bass_kernel_spmd

F32 = mybir.dt.float32
BF16 = mybir.dt.bfloat16

B, T, DIM = 2, 2048, 1024
H, HD = 16, 64
SCALE = 0.12
ROPE_BASE = 1024.0
EPS = 1e-6
G = 4          # head-groups = cores per batch
HPG = 4        # heads per group
GD = HPG * HD  # 256 dims per group
NT = T // 128  # 16 s/t tiles of 128
NJ = T // 512  # 4 t-blocks of 512

TRACE = False          # set by test.py for profiling runs
DBG = False
LAST_EXEC_NS = None    # filled when TRACE


class _TileContextFixed(tile.TileContext):
    """Workaround for this container's walrus build: the kernel-tail InstDrain
    may carry only one sync wait. Spread the tail waits over single-wait NOPs
    on the sync engine before a wait-free drain."""

    def _drain_and_barrier(self, tick_clock, wait_clock):
        nc = self.nc
        collector = nc.sync.nop(nofuse=True, hint="tail_wait_collector")
        wait_clock.add_sem_waits(
            collector.ins, ScopedClock({None: tick_clock.global_clock})
        )
        si = collector.ins.sync_info
        waits = list(si.on_wait or [])
        if len(waits) > 1:
            si.on_wait = waits[:1]
            for w in waits[1:]:
                extra = nc.sync.nop(nofuse=True, hint="tail_wait")
                esi = extra.ins.sync_info
                if esi is None:
                    extra.ins.sync_info = mybir.SyncInfo(on_wait=[w], on_update=[])
                else:
                    esi.on_wait = [w]
        nc.sync.drain()
        nc.all_engine_barrier()
        assert self.sems is not None
        popped = nc._tile_sem_poison_stack.pop()
        assert popped is self._sem_poison
        nc.clear_and_free_semaphores(list(self.sems.allocated().values()))
        nc.all_engine_barrier()


def _split_excess_waits(nc, max_waits=1):
    """This container's walrus build rejects instructions carrying more than
    one embedded sync wait. Move excess waits onto dedicated NOPs inserted
    just before the instruction on the same engine."""
    ctr = [0]
    for func in nc.m.functions:
        for block in func.blocks:
            out = []
            for inst in block.instructions:
                si = inst.sync_info
                waits = list(si.on_wait) if si and si.on_wait else []
                limit = 0 if isinstance(inst, mybir.InstDrain) else max_waits
                if len(waits) > limit:
                    keep = waits[:limit]
                    extra = waits[limit:]
                    for w in extra:
                        ctr[0] += 1
                        nop = mybir.InstNoOp(
                            name=f"waitnop-{ctr[0]}",
                            sync_info=mybir.SyncInfo(on_wait=[w], on_update=[]),
                            bass_nofuse=True,
                            engine=inst.engine,
                        )
                        out.append(nop)
                    si.on_wait = keep
                out.append(inst)
            block.instructions = out


def _rope_tables():
    keep = HD // 4  # 16 active frequencies; dims 16:32 of each half are identity
    active = (1.0 / ROPE_BASE) ** np.linspace(0.0, 1.0, keep, dtype=np.float32)
    th = np.arange(T, dtype=np.float32)[:, None] * active[None, :]
    return np.cos(th).astype(np.float32), np.sin(th).astype(np.float32)


def _classify_blocks(mask):
    """mask [T,T] additive, indexed (t, s). Block = (s-tile i of 128) x
    (t-block jb of 512). Returns cls[i][jb] in {skip, pass, tri, gen}."""
    cls = []
    for i in range(NT):
        row = []
        for jb in range(NJ):
            blk = mask[512 * jb:512 * (jb + 1), 128 * i:128 * (i + 1)]
            big_neg = blk <= -1e8
            zero = blk == 0.0
            if big_neg.all():
                row.append("skip")
            elif zero.all():
                row.append("pass")
            elif (big_neg | zero).all():
                tt = np.arange(512 * jb, 512 * (jb + 1))[:, None]
                ss = np.arange(128 * i, 128 * (i + 1))[None, :]
                row.append("tri" if np.array_equal(zero, tt >= ss) else "gen")
            else:
                row.append("gen")
        cls.append(row)
    return cls


def _build_program(cls, has_gen):
    nc = bass.Bass()
    xP = nc.declare_dram_parameter("xP", [128, NT, 8, 128], BF16, isOutput=False)
    wqP = nc.declare_dram_parameter("wqP", [128, 8, 3 * GD], BF16, isOutput=False)
    woP = nc.declare_dram_parameter("woP", [128, 2, DIM], BF16, isOutput=False)
    veP = nc.declare_dram_parameter("veP", [128, NT, GD], BF16, isOutput=False)
    lamP = nc.declare_dram_parameter("lamP", [128, 1], F32, isOutput=False)
    rcP = nc.declare_dram_parameter("rcP", [128, NT, 16], BF16, isOutput=False)
    rsP = nc.declare_dram_parameter("rsP", [128, NT, 16], BF16, isOutput=False)
    triP = nc.declare_dram_parameter("triP", [128, 128], BF16, isOutput=False)
    md = None
    if has_gen:
        md = nc.declare_dram_parameter("maskdiv", [T, T], F32, isOutput=False)
    outP = nc.declare_dram_parameter("outP", [DIM, T], BF16, isOutput=True)
    rec_dram = nc.dram_tensor("rec_scratch", [HPG, T], BF16)

    # per-jb: first/last valid s-tile for AV accumulation start/stop
    first_i = [None] * NJ
    last_i = [None] * NJ
    for jb in range(NJ):
        valid = [i for i in range(NT) if cls[i][jb] != "skip"]
        if valid:
            first_i[jb] = valid[0]
            last_i[jb] = valid[-1]

    def block_trim(i, jb):
        if cls[i][jb] != "tri":
            return 0
        tr = 128 * (i - 4 * jb)
        # the first accumulating matmul must initialize the full 512 region
        if i == first_i[jb]:
            return 0
        return tr

    with _TileContextFixed(nc) as tc, ExitStack() as ctx:
        S = ctx.enter_context(tc.tile_pool(name="singles", bufs=1))

        # ---- SBUF singles
        x_sb = S.tile([128, NT, 8, 128], BF16, tag="x_sb")
        wq_sb = S.tile([128, 8, 3 * GD], BF16, tag="wq_sb")
        wo_sb = S.tile([128, 2, DIM], BF16, tag="wo_sb")
        ve_sb = S.tile([128, NT, GD], BF16, tag="ve_sb")
        lam_sb = S.tile([128, 1], F32, tag="lam_sb")
        rc_sb = S.tile([128, NT, 16], BF16, tag="rc_sb")
        rs_sb = S.tile([128, NT, 16], BF16, tag="rs_sb")
        trikeep_sb = S.tile([128, 128], BF16, tag="trikeep_sb")

        ident = S.tile([128, 128], BF16, tag="ident")
        from concourse.masks import make_identity
        make_identity(nc, ident)

        qkv_sb = S.tile([128, NT, 3 * GD], BF16, tag="qkv_sb")
        qkT = S.tile([128, 4, T], BF16, tag="qkT")
        yT = S.tile([128, 2, T], BF16, tag="yT")
        scl_sb = S.tile([128, NT, HPG], F32, tag="scl_sb")
        rstdq = S.tile([128, NT, 12], F32, tag="rstdq")
        rstdv = S.tile([128, NT, HPG], F32, tag="rstdv")
        ms = S.tile([128, NT, 12], F32, tag="ms")

        # ---- input DMAs.  queue SP: x tiles (prefetch-ordered; more emitted
        # inside emit_tile).  queue Act: weights + small tables.
        for ds in range(8):
            nc.scalar.dma_start(out=wq_sb[:, ds, :], in_=wqP[:, ds, :])
        x_fetched = [False] * NT

        def fetch_x(tt):
            if not x_fetched[tt]:
                x_fetched[tt] = True
                nc.sync.dma_start(out=x_sb[:, tt], in_=xP[:, tt])

        for tt in range(6):
            fetch_x(tt)
        nc.sync.dma_start(out=lam_sb, in_=lamP[:, :])
        nc.sync.dma_start(out=rc_sb, in_=rcP[:, :, :])
        nc.sync.dma_start(out=rs_sb, in_=rsP[:, :, :])
        nc.sync.dma_start(out=trikeep_sb, in_=triP[:, :])
        nc.scalar.dma_start(out=ve_sb, in_=veP[:, :, :])
        nc.scalar.dma_start(out=wo_sb, in_=woP[:, :, :])

        # v_aug[p, tt, h, 0:128]: even h -> [v | ones], odd h -> [ones | v]
        v_aug = S.tile([128, NT, HPG, 128], BF16, tag="v_aug")
        v5 = v_aug.rearrange("p t (a b) c -> p t a b c", b=2)
        nc.gpsimd.memset(v5[:, :, :, 0, 64:128], 1.0)
        nc.gpsimd.memset(v5[:, :, :, 1, 0:64], 1.0)

        # ---- pools
        PS = ctx.enter_context(tc.tile_pool(name="ps", bufs=2, space="PSUM"))
        EX = ctx.enter_context(tc.tile_pool(name="ex_sb", bufs=20))
        A = ctx.enter_context(tc.tile_pool(name="a_sb", bufs=2))
        NR = ctx.enter_context(tc.tile_pool(name="rec_sb", bufs=2))
        OS = ctx.enter_context(tc.tile_pool(name="os_sb", bufs=3))
        MD = ctx.enter_context(tc.tile_pool(name="md_sb", bufs=2))

        # ================= stage A =================
        sqs = {}
        pending_back = []
        pending_tp = []

        def emit_tile_front(tt):
            """QKV matmuls + raw copy + square for t-tile tt."""
            fetch_x(min(tt + 6, NT - 1))
            qp = PS.tile([128, 1024], F32, tag="sc", name="qp")
            for ds in range(8):
                nc.tensor.matmul(
                    qp[:, 0:512], x_sb[:, tt, ds, :], wq_sb[:, ds, 0:512],
                    start=(ds == 0), stop=(ds == 7))
            for ds in range(8):
                nc.tensor.matmul(
                    qp[:, 512:768], x_sb[:, tt, ds, :], wq_sb[:, ds, 512:768],
                    start=(ds == 0), stop=(ds == 7))
            nc.scalar.activation(
                qkv_sb[:, tt, :], qp[:, 0:768],
                mybir.ActivationFunctionType.Copy)
            sq = A.tile([128, 768], BF16, tag="sq", name="sq")
            sqs[tt] = sq
            nc.gpsimd.tensor_tensor(
                sq, qkv_sb[:, tt, :], qkv_sb[:, tt, :], mybir.AluOpType.mult)

        def emit_tile_back(tt):
            """stat chain + norm + rope + v-blend for t-tile tt (runs one
            filler period after the front so no engine head-of-line blocks)."""
            sq = sqs.pop(tt)
            nc.vector.tensor_reduce(
                ms[:, tt, :],
                sq.rearrange("p (g d) -> p g d", d=HD),
                axis=mybir.AxisListType.X, op=mybir.AluOpType.add)
            hsl = slice(tt, tt + 1)
            mm = A.tile([128, 1, 12], F32, tag="mm", name="mm")
            nc.vector.tensor_scalar(
                out=mm, in0=ms[:, hsl, :], scalar1=1.0 / HD, scalar2=EPS,
                op0=mybir.AluOpType.mult, op1=mybir.AluOpType.add)
            lnm = A.tile([128, 1, 12], F32, tag="lnm", name="lnm")
            nc.scalar.activation(
                lnm, mm, mybir.ActivationFunctionType.Ln)
            nc.scalar.activation(
                rstdq[:, hsl, :], lnm, mybir.ActivationFunctionType.Exp,
                scale=-0.5)
            nc.vector.tensor_scalar_mul(
                scl_sb[:, hsl, :], rstdq[:, hsl, 4:8], SCALE)
            nc.vector.tensor_scalar_mul(
                rstdv[:, hsl, :], rstdq[:, hsl, 8:12], lam_sb[:, 0:1])

            qk4 = qkv_sb[:, hsl, :].rearrange("p t (g d) -> p t g d", d=HD)
            if has_gen:
                nc.vector.tensor_tensor(
                    qk4[:, :, 4:8, :], qk4[:, :, 4:8, :],
                    rstdq[:, hsl, 4:8, None].to_broadcast([128, 1, 4, HD]),
                    mybir.AluOpType.mult)
            nc.vector.tensor_tensor(
                qk4[:, :, 0:4, :], qk4[:, :, 0:4, :],
                rstdq[:, hsl, 0:4, None].to_broadcast([128, 1, 4, HD]),
                mybir.AluOpType.mult)

            vn = A.tile([128, 1, 4, HD], BF16, tag="vn", name="vn")
            nc.vector.tensor_tensor(
                vn, qk4[:, :, 8:12, :],
                rstdv[:, hsl, :, None].to_broadcast([128, 1, 4, HD]),
                mybir.AluOpType.mult)
            vn4 = vn.rearrange("p t (a b) d -> p t a b d", b=2)
            vev = ve_sb[:, hsl, :].rearrange(
                "p t (a b d) -> p t a b d", a=2, d=HD)
            for a in range(2):
                nc.gpsimd.tensor_tensor(
                    v5[:, hsl, a, 0, 0:64], vn4[:, :, a, 0, :],
                    vev[:, :, a, 0, :], mybir.AluOpType.add)
                nc.gpsimd.tensor_tensor(
                    v5[:, hsl, a, 1, 64:128], vn4[:, :, a, 1, :],
                    vev[:, :, a, 1, :], mybir.AluOpType.add)

            v6 = qkv_sb[:, hsl, 0:512].rearrange(
                "p t (sg d) -> p t sg d", d=HD)
            x0 = v6[:, :, :, 0:16]
            x32 = v6[:, :, :, 32:48]
            cb = rc_sb[:, hsl, None, :].to_broadcast([128, 1, 8, 16])
            sb = rs_sb[:, hsl, None, :].to_broadcast([128, 1, 8, 16])
            ra = A.tile([128, 1, 8, 16], BF16, tag="ra", name="ra")
            rb = A.tile([128, 1, 8, 16], BF16, tag="rb", name="rb")
            nc.gpsimd.tensor_tensor(ra, x0, sb, mybir.AluOpType.mult)
            nc.gpsimd.tensor_tensor(rb, x32, sb, mybir.AluOpType.mult)
            nc.vector.tensor_tensor(x0, x0, cb, mybir.AluOpType.mult)
            nc.vector.tensor_tensor(x32, x32, cb, mybir.AluOpType.mult)
            nc.vector.tensor_tensor(x0, x0, rb, mybir.AluOpType.add)
            nc.vector.tensor_tensor(x32, x32, ra, mybir.AluOpType.subtract)

        def emit_transposes(tt):
            # q,k transposes via the DMA XBAR on the sync queue (the
            # descriptor generation occupies the sync engine ~1.2us per
            # chunk; keeps the PE free)
            for ec in range(4):
                nc.sync.dma_start(
                    out=qkT[:, ec, 128 * tt:128 * (tt + 1)],
                    in_=qkv_sb[:, tt, 128 * ec:128 * (ec + 1)],
                    transpose=True)

        def emit_tile(tt):
            emit_tile_front(tt)
            emit_tile_back(tt)
            pending_tp.append(tt)
            if len(pending_tp) > 2:
                emit_transposes(pending_tp.pop(0))

        def flush_transposes():
            while pending_tp:
                emit_transposes(pending_tp.pop(0))

        # ================= stage B =================
        # Block-level software pipeline: while the PE runs the AV matmuls of
        # block X-1 (whose exps finished a block ago, buffered in SBUF ex
        # tiles), it interleaves the scores matmuls of block X and the Act
        # engine streams block X's exps.  The exp latency is thus never on
        # the PE's critical path, and the PE stays dense (HAM stays warm).
        exs = {}
        avs = {}

        def ivals_of(h, jj):
            jbsel = (2 * jj, 2 * jj + 1)
            return [i for i in range(NT)
                    if any(cls[i][jb] != "skip" for jb in jbsel)]

        def jbs_of(i, jj):
            return [jb for jb in (2 * jj, 2 * jj + 1) if cls[i][jb] != "skip"]

        def emit_sc_iter(h, jj, i):
            par, pair = h % 2, h // 2
            rlo, rhi = 64 * par, 64 * par + 64
            sc = PS.tile([128, 1024], F32, tag="sc", name="sc")
            jbs = jbs_of(i, jj)
            for jb in jbs:
                off = 512 * (jb - 2 * jj)
                trim = block_trim(i, jb)
                nc.tensor.matmul(
                    sc[:, off + trim:off + 512],
                    qkT[rlo:rhi, 2 + pair, 128 * i:128 * (i + 1)],
                    qkT[rlo:rhi, pair, 512 * jb + trim:512 * (jb + 1)],
                    start=True, stop=True)
                if cls[i][jb] == "gen":
                    mdt = MD.tile([128, 512], F32, tag="mdt", name="mdt")
                    nc.sync.dma_start(
                        out=mdt,
                        in_=md[512 * jb:512 * (jb + 1),
                               128 * i:128 * (i + 1)].rearrange("t s -> s t"))
                    nc.vector.tensor_tensor(
                        sc[:, off:off + 512], sc[:, off:off + 512],
                        mdt, mybir.AluOpType.add)
            ex = EX.tile([128, 1024], BF16, tag="ex", name="ex")
            exs[(h, jj, i)] = ex
            lo = 512 * (min(jbs) - 2 * jj) + block_trim(i, min(jbs))
            hi = 512 * (max(jbs) - 2 * jj) + 512
            escale = (SCALE if has_gen else scl_sb[:, i, h:h + 1])
            nc.scalar.activation(
                ex[:, lo:hi], sc[:, lo:hi],
                mybir.ActivationFunctionType.Exp, scale=escale)
            for jb in jbs:
                if cls[i][jb] != "tri":
                    continue
                # zero the above-diagonal half of the 128-wide strip
                # (gpsimd; consumed by the AV matmuls a block later)
                off = 512 * (jb - 2 * jj)
                strip = off + 128 * (i - 4 * jb)
                nc.gpsimd.tensor_tensor(
                    ex[:, strip:strip + 128],
                    ex[:, strip:strip + 128],
                    trikeep_sb, mybir.AluOpType.mult)

        def emit_av_iter(h, jj, jb, i):
            if (h, jj) not in avs:
                avs[(h, jj)] = PS.tile([128, 1024], F32, tag="av", name="av")
            av = avs[(h, jj)]
            ex = exs[(h, jj, i)]
            off = 512 * (jb - 2 * jj)
            trim = block_trim(i, jb)
            nc.tensor.matmul(
                av[:, off + trim:off + 512],
                v_aug[:, i, h, :],
                ex[:, off + trim:off + 512],
                start=(first_i[jb] == i),
                stop=(last_i[jb] == i))

        def emit_block_tail(h, jj):
            """den extraction + reciprocal + broadcast + y normalize."""
            par, pair = h % 2, h // 2
            rlo, rhi = 64 * par, 64 * par + 64
            dlo = 64 - rlo
            av = avs.pop((h, jj))
            for i in ivals_of(h, jj):
                exs.pop((h, jj, i), None)
            for jb in (2 * jj, 2 * jj + 1):
                if first_i[jb] is None:
                    off = 512 * (jb - 2 * jj)
                    nc.vector.memset(av[:, off:off + 512], 1.0)
            den_sb = NR.tile([1, 1024], F32, tag="den_sb", name="den_sb")
            nc.vector.tensor_scalar_mul(den_sb, av[dlo:dlo + 1, :], 1.0)
            den_pk = NR.tile([128, 8], F32, tag="den_pk", name="den_pk")
            nc.scalar.dma_start(out=den_pk, in_=den_sb)
            rec_pk = NR.tile([128, 8], F32, tag="rec_pk", name="rec_pk")
            nc.vector.reciprocal(rec_pk, den_pk)
            rec_bf = NR.tile([128, 8], BF16, tag="rec_bf", name="rec_bf")
            nc.vector.tensor_copy(out=rec_bf, in_=rec_pk)
            hsl2 = slice(1024 * jj, 1024 * (jj + 1))
            nc.scalar.dma_start(out=rec_dram[h, hsl2], in_=rec_bf)
            rrow_ap = rec_dram[h, hsl2]
            rec_bc = bass.AP(
                tensor=rrow_ap.tensor,
                offset=rrow_ap.offset,
                ap=[[0, 64]] + [list(p) for p in rrow_ap.ap])
            rec = NR.tile([64, 1024], BF16, tag="rec", name="rec")
            nc.scalar.dma_start(out=rec, in_=rec_bc)
            nc.vector.tensor_tensor(
                yT[rlo:rhi, pair, hsl2],
                av[rlo:rhi, :], rec,
                mybir.AluOpType.mult)

        def run_block_pair(blk, prev, fillers, cadence):
            """Interleave scores/exp of `blk` with AVs of `prev`."""
            sc_ops = ([lambda h=blk[0], jj=blk[1], i=i: emit_sc_iter(h, jj, i)
                       for i in ivals_of(*blk)] if blk else [])
            av_ops = []
            if prev:
                hP, jjP = prev
                for jb in (2 * jjP, 2 * jjP + 1):
                    for i in range(NT):
                        if cls[i][jb] != "skip":
                            av_ops.append(
                                lambda h=hP, jj=jjP, jb=jb, i=i:
                                emit_av_iter(h, jj, jb, i))
            seq = ([(((k + 0.5) / len(sc_ops)), 0, f)
                    for k, f in enumerate(sc_ops)]
                   + [(((k + 0.5) / len(av_ops)), 1, f)
                      for k, f in enumerate(av_ops)])
            seq.sort(key=lambda t: (t[0], t[1]))
            for n, (_, _, f) in enumerate(seq):
                f()
                if fillers and n % cadence == cadence - 1:
                    fillers.pop(0)()
            if prev:
                emit_block_tail(*prev)

        # ================= stage C =================
        def emit_po(ec, tb):
            po = PS.tile([128, 1024], F32, tag="sc", name="po")
            for dc in range(2):
                nc.tensor.matmul(
                    po[:, 0:512],
                    wo_sb[:, dc, 128 * ec:128 * (ec + 1)],
                    yT[:, dc, 512 * tb:512 * (tb + 1)],
                    start=(dc == 0), stop=(dc == 1))
            os = OS.tile([128, 512], BF16, tag="os", name="os")
            nc.vector.tensor_copy(out=os, in_=po[:, 0:512])
            nc.sync.dma_start(
                out=outP[128 * ec:128 * (ec + 1), 512 * tb:512 * (tb + 1)],
                in_=os)

        # ================= emission =================
        for tt in range(10):
            emit_tile(tt)
        flush_transposes()

        fillers = [lambda tt=tt: emit_tile(tt) for tt in range(10, 16)]
        fillers.append(flush_transposes)
        blocks = [(h, 0) for h in range(HPG)] + [(h, 1) for h in range(HPG)]
        po_started = [False]

        def add_po_fillers():
            if not po_started[0]:
                po_started[0] = True
                for ec in range(8):
                    for tb in (0, 1):
                        fillers.append(
                            lambda ec=ec, tb=tb: emit_po(ec, tb))

        prev = None
        for bi, blk in enumerate(blocks):
            if bi == HPG:
                flush_transposes()
            if bi == HPG + 1:
                # all jj0 tails are emitted once prev==(h3,0) is retired
                add_po_fillers()
            run_block_pair(blk, prev, fillers, 3 if bi < 2 else 5)
            prev = blk
        add_po_fillers()
        run_block_pair(None, prev, fillers, 3)

        while fillers:
            fillers.pop(0)()
        for ec in range(8):
            for tb in (2, 3):
                emit_po(ec, tb)
    _split_excess_waits(nc)
    return nc


def kernel(x, ve, sa_lambdas, attn_mask, qkvo_w):
    global LAST_EXEC_NS
    x = np.ascontiguousarray(np.asarray(x, np.float32))
    ve = np.ascontiguousarray(np.asarray(ve, np.float32))
    sa_lambdas = np.asarray(sa_lambdas, np.float32)
    attn_mask = np.asarray(attn_mask, np.float32)
    qkvo_w = np.asarray(qkvo_w, np.float32)

    ropeC, ropeS = _rope_tables()
    mask = attn_mask[0, 0]
    cls = _classify_blocks(mask)
    for jb in range(NJ):
        valid = [i for i in range(NT) if cls[i][jb] != "skip"]
        if valid and cls[valid[0]][jb] == "tri" and valid[0] - 4 * jb > 0:
            # the tri fast path assumes the strip starts inside the block
            cls[valid[0]][jb] = "gen"
    has_gen = any(c == "gen" for row in cls for c in row)

    nc = _build_program(cls, has_gen)

    part = np.arange(128)
    trikeep = (part[:, None] <= part[None, :]).astype(np.float32)      # [p, c]
    trikeep = np.ascontiguousarray(trikeep).astype(ml_dtypes.bfloat16)
    lam0 = np.full((128, 1), sa_lambdas[0], np.float32)
    rcP = np.ascontiguousarray(
        ropeC.reshape(NT, 128, 16).transpose(1, 0, 2)).astype(ml_dtypes.bfloat16)
    rsP = np.ascontiguousarray(
        ropeS.reshape(NT, 128, 16).transpose(1, 0, 2)).astype(ml_dtypes.bfloat16)
    maskdiv = (mask / SCALE).astype(np.float32) if has_gen else None

    in_maps = []
    for c in range(8):
        b, g = c // G, c % G
        sl = slice(GD * g, GD * (g + 1))
        wqkvT = np.concatenate([qkvo_w[k][sl, :] for k in range(3)], 0).T
        # xP[p, tt, ds, c] = x[b][128*tt + c, 128*ds + p]
        xco = np.ascontiguousarray(
            x[b].T.reshape(8, 128, NT, 128).transpose(1, 2, 0, 3)
        ).astype(ml_dtypes.bfloat16)
        m = {
            "xP": xco,
            "wqP": np.ascontiguousarray(
                wqkvT.reshape(8, 128, 3 * GD).transpose(1, 0, 2)
            ).astype(ml_dtypes.bfloat16),
            "woP": np.ascontiguousarray(
                qkvo_w[3][:, sl].T.reshape(2, 128, DIM).transpose(1, 0, 2)
            ).astype(ml_dtypes.bfloat16),
            "veP": np.ascontiguousarray(
                (ve[b][:, sl] * sa_lambdas[1]).reshape(NT, 128, GD)
                .transpose(1, 0, 2)
            ).astype(ml_dtypes.bfloat16),
            "lamP": lam0,
            "rcP": rcP,
            "rsP": rsP,
            "triP": trikeep,
        }
        if has_gen:
            m["maskdiv"] = maskdiv
        in_maps.append(m)

    res = run_bass_kernel_spmd(nc, in_maps, core_ids=list(range(8)),
                               trace=TRACE)
    if TRACE:
        LAST_EXEC_NS = res.exec_time_ns

    out = np.zeros((B, T, DIM), np.float32)
    for c in range(8):
        out[c // G] += res.results[c]["outP"].astype(np.float32).T
    return out


# revision 20
# speedup vs baseline: 1.0608x; 1.0167x over previous
"""Trainium2 Bass kernel for nn_CausalSelfAttention_28467043237962.

Sharding: 8 cores = 2 batches x 4 head-groups (4 heads / 256 dims each).

v2 design (vs the 235us baseline):
- t-tile-major QKV: one [128,1024]f32 PSUM slot per t-tile (two chunk writes),
  raw bf16 copy to SBUF, squares/reduce for RMS stats.
- rstd via exp(-0.5*ln(m)) on the scalar engine: Ln/Exp/Copy all live in one
  activation-table set, so the table loads once (no Sqrt thrashing).  The
  SCALE and lambda0 factors ride the exp bias (ln SCALE / ln lambda0).
- q/k transposes via the DMA XBAR (dma_start transpose=True), not the PE.
- Attention in transposed-scores layout (scores_T[s,t]); softmax denominators
  from ones-columns inside the AV matmul; per (h, jj-half) the i-loop is
  software-pipelined (sc_{i+1} emitted before av_i) so the Act exp latency
  hides behind PE work.
- PE fillers keep the tensor engine dense (HAM stays at K=8/8): QKV quarters
  2,3 are emitted inside attention block jj=0; the jj=0 output projection is
  emitted inside attention block jj=1.
- AV matmuls and exp are trimmed at the causal diagonal; the tri mask
  multiply only touches the 128-wide diagonal strip (gpsimd engine).
- PSUM: tag "sc" [128,1024]f32 x2 shared by scores / QKV / out-proj chunks,
  tag "av" [128,1024]f32 x2.  16KB exactly.

Self-contained: builds one SPMD Bass program and runs it on cores 0-7 via
concourse.bass_utils.run_bass_kernel_spmd.
"""
import sys

sys.path.insert(0, "/opt/trn_rl_repo")

from contextlib import ExitStack

import numpy as np
import ml_dtypes

import concourse.bass as bass
import concourse.tile as tile
import concourse.mybir as mybir
from concourse.vector_clock import ScopedClock
from concourse.bass_utils import run_bass_kernel_spmd

F32 = mybir.dt.float32
BF16 = mybir.dt.bfloat16

B, T, DIM = 2, 2048, 1024
H, HD = 16, 64
SCALE = 0.12
ROPE_BASE = 1024.0
EPS = 1e-6
G = 4          # head-groups = cores per batch
HPG = 4        # heads per group
GD = HPG * HD  # 256 dims per group
NT = T // 128  # 16 s/t tiles of 128
NJ = T // 512  # 4 t-blocks of 512

TRACE = False          # set by test.py for profiling runs
DBG = False
LAST_EXEC_NS = None    # filled when TRACE


class _TileContextFixed(tile.TileContext):
    """Workaround for this container's walrus build: the kernel-tail InstDrain
    may carry only one sync wait. Spread the tail waits over single-wait NOPs
    on the sync engine before a wait-free drain."""

    def _drain_and_barrier(self, tick_clock, wait_clock):
        nc = self.nc
        collector = nc.sync.nop(nofuse=True, hint="tail_wait_collector")
        wait_clock.add_sem_waits(
            collector.ins, ScopedClock({None: tick_clock.global_clock})
        )
        si = collector.ins.sync_info
        waits = list(si.on_wait or [])
        if len(waits) > 1:
            si.on_wait = waits[:1]
            for w in waits[1:]:
                extra = nc.sync.nop(nofuse=True, hint="tail_wait")
                esi = extra.ins.sync_info
                if esi is None:
                    extra.ins.sync_info = mybir.SyncInfo(on_wait=[w], on_update=[])
                else:
                    esi.on_wait = [w]
        nc.sync.drain()
        nc.all_engine_barrier()
        assert self.sems is not None
        popped = nc._tile_sem_poison_stack.pop()
        assert popped is self._sem_poison
        nc.clear_and_free_semaphores(list(self.sems.allocated().values()))
        nc.all_engine_barrier()


def _split_excess_waits(nc, max_waits=1):
    """This container's walrus build rejects instructions carrying more than
    one embedded sync wait. Move excess waits onto dedicated NOPs inserted
    just before the instruction on the same engine."""
    ctr = [0]
    for func in nc.m.functions:
        for block in func.blocks:
            out = []
            for inst in block.instructions:
                si = inst.sync_info
                waits = list(si.on_wait) if si and si.on_wait else []
                limit = 0 if isinstance(inst, mybir.InstDrain) else max_waits
                if len(waits) > limit:
                    keep = waits[:limit]
                    extra = waits[limit:]
                    for w in extra:
                        ctr[0] += 1
                        nop = mybir.InstNoOp(
                            name=f"waitnop-{ctr[0]}",
                            sync_info=mybir.SyncInfo(on_wait=[w], on_update=[]),
                            bass_nofuse=True,
                            engine=inst.engine,
                        )
                        out.append(nop)
                    si.on_wait = keep
                out.append(inst)
            block.instructions = out


def _rope_tables():
    keep = HD // 4  # 16 active frequencies; dims 16:32 of each half are identity
    active = (1.0 / ROPE_BASE) ** np.linspace(0.0, 1.0, keep, dtype=np.float32)
    th = np.arange(T, dtype=np.float32)[:, None] * active[None, :]
    return np.cos(th).astype(np.float32), np.sin(th).astype(np.float32)


def _classify_blocks(mask):
    """mask [T,T] additive, indexed (t, s). Block = (s-tile i of 128) x
    (t-block jb of 512). Returns cls[i][jb] in {skip, pass, tri, gen}."""
    cls = []
    for i in range(NT):
        row = []
        for jb in range(NJ):
            blk = mask[512 * jb:512 * (jb + 1), 128 * i:128 * (i + 1)]
            big_neg = blk <= -1e8
            zero = blk == 0.0
            if big_neg.all():
                row.append("skip")
            elif zero.all():
                row.append("pass")
            elif (big_neg | zero).all():
                tt = np.arange(512 * jb, 512 * (jb + 1))[:, None]
                ss = np.arange(128 * i, 128 * (i + 1))[None, :]
                row.append("tri" if np.array_equal(zero, tt >= ss) else "gen")
            else:
                row.append("gen")
        cls.append(row)
    return cls


def _build_program(cls, has_gen):
    nc = bass.Bass()
    xP = nc.declare_dram_parameter("xP", [128, NT, 8, 128], BF16, isOutput=False)
    wqP = nc.declare_dram_parameter("wqP", [128, 8, 3 * GD], BF16, isOutput=False)
    woP = nc.declare_dram_parameter("woP", [128, 2, DIM], BF16, isOutput=False)
    veP = nc.declare_dram_parameter("veP", [128, NT, GD], BF16, isOutput=False)
    lamP = nc.declare_dram_parameter("lamP", [128, 1], F32, isOutput=False)
    rcP = nc.declare_dram_parameter("rcP", [128, NT, 16], BF16, isOutput=False)
    rsP = nc.declare_dram_parameter("rsP", [128, NT, 16], BF16, isOutput=False)
    triP = nc.declare_dram_parameter("triP", [128, 128], BF16, isOutput=False)
    md = None
    if has_gen:
        md = nc.declare_dram_parameter("maskdiv", [T, T], F32, isOutput=False)
    outP = nc.declare_dram_parameter("outP", [DIM, T], BF16, isOutput=True)
    rec_dram = nc.dram_tensor("rec_scratch", [HPG, T], BF16)

    # per-jb: first/last valid s-tile for AV accumulation start/stop
    first_i = [None] * NJ
    last_i = [None] * NJ
    for jb in range(NJ):
        valid = [i for i in range(NT) if cls[i][jb] != "skip"]
        if valid:
            first_i[jb] = valid[0]
            last_i[jb] = valid[-1]

    def block_trim(i, jb):
        if cls[i][jb] != "tri":
            return 0
        tr = 128 * (i - 4 * jb)
        # the first accumulating matmul must initialize the full 512 region
        if i == first_i[jb]:
            return 0
        return tr

    with _TileContextFixed(nc) as tc, ExitStack() as ctx:
        S = ctx.enter_context(tc.tile_pool(name="singles", bufs=1))

        # ---- SBUF singles
        x_sb = S.tile([128, NT, 8, 128], BF16, tag="x_sb")
        wq_sb = S.tile([128, 8, 3 * GD], BF16, tag="wq_sb")
        wo_sb = S.tile([128, 2, DIM], BF16, tag="wo_sb")
        ve_sb = S.tile([128, NT, GD], BF16, tag="ve_sb")
        lam_sb = S.tile([128, 1], F32, tag="lam_sb")
        rc_sb = S.tile([128, NT, 16], BF16, tag="rc_sb")
        rs_sb = S.tile([128, NT, 16], BF16, tag="rs_sb")
        trikeep_sb = S.tile([128, 128], BF16, tag="trikeep_sb")

        ident = S.tile([128, 128], BF16, tag="ident")
        from concourse.masks import make_identity
        make_identity(nc, ident)

        qkv_sb = S.tile([128, NT, 3 * GD], BF16, tag="qkv_sb")
        qkT = S.tile([128, 4, T], BF16, tag="qkT")
        yT = S.tile([128, 2, T], BF16, tag="yT")
        scl_sb = S.tile([128, NT, HPG], F32, tag="scl_sb")
        rstdq = S.tile([128, NT, 12], F32, tag="rstdq")
        rstdv = S.tile([128, NT, HPG], F32, tag="rstdv")
        ms = S.tile([128, NT, 12], F32, tag="ms")

        # ---- input DMAs.  queue SP: x tiles (prefetch-ordered; more emitted
        # inside emit_tile).  queue Act: weights + small tables.
        for ds in range(8):
            nc.scalar.dma_start(out=wq_sb[:, ds, :], in_=wqP[:, ds, :])
        x_fetched = [False] * NT

        def fetch_x(tt):
            if not x_fetched[tt]:
                x_fetched[tt] = True
                nc.sync.dma_start(out=x_sb[:, tt], in_=xP[:, tt])

        for tt in range(6):
            fetch_x(tt)
        nc.sync.dma_start(out=lam_sb, in_=lamP[:, :])
        nc.sync.dma_start(out=rc_sb, in_=rcP[:, :, :])
        nc.sync.dma_start(out=rs_sb, in_=rsP[:, :, :])
        nc.sync.dma_start(out=trikeep_sb, in_=triP[:, :])
        nc.scalar.dma_start(out=ve_sb, in_=veP[:, :, :])
        nc.scalar.dma_start(out=wo_sb, in_=woP[:, :, :])

        # v_aug[p, tt, h, 0:128]: even h -> [v | ones], odd h -> [ones | v]
        v_aug = S.tile([128, NT, HPG, 128], BF16, tag="v_aug")
        v5 = v_aug.rearrange("p t (a b) c -> p t a b c", b=2)
        nc.gpsimd.memset(v5[:, :, :, 0, 64:128], 1.0)
        nc.gpsimd.memset(v5[:, :, :, 1, 0:64], 1.0)

        # ---- pools
        PS = ctx.enter_context(tc.tile_pool(name="ps", bufs=2, space="PSUM"))
        EX = ctx.enter_context(tc.tile_pool(name="ex_sb", bufs=20))
        A = ctx.enter_context(tc.tile_pool(name="a_sb", bufs=2))
        NR = ctx.enter_context(tc.tile_pool(name="rec_sb", bufs=2))
        OS = ctx.enter_context(tc.tile_pool(name="os_sb", bufs=3))
        MD = ctx.enter_context(tc.tile_pool(name="md_sb", bufs=2))

        # ================= stage A =================
        sqs = {}
        pending_back = []
        pending_tp = []

        def emit_tile_front(tt):
            """QKV matmuls + raw copy + square for t-tile tt."""
            fetch_x(min(tt + 6, NT - 1))
            qp = PS.tile([128, 1024], F32, tag="sc", name="qp")
            for ds in range(8):
                nc.tensor.matmul(
                    qp[:, 0:512], x_sb[:, tt, ds, :], wq_sb[:, ds, 0:512],
                    start=(ds == 0), stop=(ds == 7))
            for ds in range(8):
                nc.tensor.matmul(
                    qp[:, 512:768], x_sb[:, tt, ds, :], wq_sb[:, ds, 512:768],
                    start=(ds == 0), stop=(ds == 7))
            nc.scalar.activation(
                qkv_sb[:, tt, :], qp[:, 0:768],
                mybir.ActivationFunctionType.Copy)
            sq = A.tile([128, 768], BF16, tag="sq", name="sq")
            sqs[tt] = sq
            nc.gpsimd.tensor_tensor(
                sq, qkv_sb[:, tt, :], qkv_sb[:, tt, :], mybir.AluOpType.mult)

        def emit_tile_back(tt):
            """stat chain + norm + rope + v-blend for t-tile tt (runs one
            filler period after the front so no engine head-of-line blocks)."""
            sq = sqs.pop(tt)
            nc.vector.tensor_reduce(
                ms[:, tt, :],
                sq.rearrange("p (g d) -> p g d", d=HD),
                axis=mybir.AxisListType.X, op=mybir.AluOpType.add)
            hsl = slice(tt, tt + 1)
            mm = A.tile([128, 1, 12], F32, tag="mm", name="mm")
            nc.vector.tensor_scalar(
                out=mm, in0=ms[:, hsl, :], scalar1=1.0 / HD, scalar2=EPS,
                op0=mybir.AluOpType.mult, op1=mybir.AluOpType.add)
            lnm = A.tile([128, 1, 12], F32, tag="lnm", name="lnm")
            nc.scalar.activation(
                lnm, mm, mybir.ActivationFunctionType.Ln)
            nc.scalar.activation(
                rstdq[:, hsl, :], lnm, mybir.ActivationFunctionType.Exp,
                scale=-0.5)
            nc.vector.tensor_scalar_mul(
                scl_sb[:, hsl, :], rstdq[:, hsl, 4:8], SCALE)
            nc.vector.tensor_scalar_mul(
                rstdv[:, hsl, :], rstdq[:, hsl, 8:12], lam_sb[:, 0:1])

            qk4 = qkv_sb[:, hsl, :].rearrange("p t (g d) -> p t g d", d=HD)
            if has_gen:
                nc.vector.tensor_tensor(
                    qk4[:, :, 4:8, :], qk4[:, :, 4:8, :],
                    rstdq[:, hsl, 4:8, None].to_broadcast([128, 1, 4, HD]),
                    mybir.AluOpType.mult)
            nc.vector.tensor_tensor(
                qk4[:, :, 0:4, :], qk4[:, :, 0:4, :],
                rstdq[:, hsl, 0:4, None].to_broadcast([128, 1, 4, HD]),
                mybir.AluOpType.mult)

            vn = A.tile([128, 1, 4, HD], BF16, tag="vn", name="vn")
            nc.vector.tensor_tensor(
                vn, qk4[:, :, 8:12, :],
                rstdv[:, hsl, :, None].to_broadcast([128, 1, 4, HD]),
                mybir.AluOpType.mult)
            vn4 = vn.rearrange("p t (a b) d -> p t a b d", b=2)
            vev = ve_sb[:, hsl, :].rearrange(
                "p t (a b d) -> p t a b d", a=2, d=HD)
            for a in range(2):
                nc.gpsimd.tensor_tensor(
                    v5[:, hsl, a, 0, 0:64], vn4[:, :, a, 0, :],
                    vev[:, :, a, 0, :], mybir.AluOpType.add)
                nc.gpsimd.tensor_tensor(
                    v5[:, hsl, a, 1, 64:128], vn4[:, :, a, 1, :],
                    vev[:, :, a, 1, :], mybir.AluOpType.add)

            v6 = qkv_sb[:, hsl, 0:512].rearrange(
                "p t (sg d) -> p t sg d", d=HD)
            x0 = v6[:, :, :, 0:16]
            x32 = v6[:, :, :, 32:48]
            cb = rc_sb[:, hsl, None, :].to_broadcast([128, 1, 8, 16])
            sb = rs_sb[:, hsl, None, :].to_broadcast([128, 1, 8, 16])
            ra = A.tile([128, 1, 8, 16], BF16, tag="ra", name="ra")
            rb = A.tile([128, 1, 8, 16], BF16, tag="rb", name="rb")
            nc.gpsimd.tensor_tensor(ra, x0, sb, mybir.AluOpType.mult)
            nc.gpsimd.tensor_tensor(rb, x32, sb, mybir.AluOpType.mult)
            nc.vector.tensor_tensor(x0, x0, cb, mybir.AluOpType.mult)
            nc.vector.tensor_tensor(x32, x32, cb, mybir.AluOpType.mult)
            nc.vector.tensor_tensor(x0, x0, rb, mybir.AluOpType.add)
            nc.vector.tensor_tensor(x32, x32, ra, mybir.AluOpType.subtract)

        def emit_transposes(tt):
            # q,k transposes via the DMA XBAR on the sync queue (the
            # descriptor generation occupies the sync engine ~1.2us per
            # chunk; keeps the PE free)
            for ec in range(4):
                nc.sync.dma_start(
                    out=qkT[:, ec, 128 * tt:128 * (tt + 1)],
                    in_=qkv_sb[:, tt, 128 * ec:128 * (ec + 1)],
                    transpose=True)

        def emit_tile(tt):
            emit_tile_front(tt)
            emit_tile_back(tt)
            pending_tp.append(tt)
            if len(pending_tp) > 2:
                emit_transposes(pending_tp.pop(0))

        def flush_transposes():
            while pending_tp:
                emit_transposes(pending_tp.pop(0))

        # ================= stage B =================
        # Block-level software pipeline: while the PE runs the AV matmuls of
        # block X-1 (whose exps finished a block ago, buffered in SBUF ex
        # tiles), it interleaves the scores matmuls of block X and the Act
        # engine streams block X's exps.  The exp latency is thus never on
        # the PE's critical path, and the PE stays dense (HAM stays warm).
        exs = {}
        avs = {}

        def ivals_of(h, jj):
            jbsel = (2 * jj, 2 * jj + 1)
            return [i for i in range(NT)
                    if any(cls[i][jb] != "skip" for jb in jbsel)]

        def jbs_of(i, jj):
            return [jb for jb in (2 * jj, 2 * jj + 1) if cls[i][jb] != "skip"]

        def emit_sc_iter(h, jj, i):
            par, pair = h % 2, h // 2
            rlo, rhi = 64 * par, 64 * par + 64
            sc = PS.tile([128, 1024], F32, tag="sc", name="sc")
            jbs = jbs_of(i, jj)
            for jb in jbs:
                off = 512 * (jb - 2 * jj)
                trim = block_trim(i, jb)
                nc.tensor.matmul(
                    sc[:, off + trim:off + 512],
                    qkT[rlo:rhi, 2 + pair, 128 * i:128 * (i + 1)],
                    qkT[rlo:rhi, pair, 512 * jb + trim:512 * (jb + 1)],
                    start=True, stop=True)
                if cls[i][jb] == "gen":
                    mdt = MD.tile([128, 512], F32, tag="mdt", name="mdt")
                    nc.sync.dma_start(
                        out=mdt,
                        in_=md[512 * jb:512 * (jb + 1),
                               128 * i:128 * (i + 1)].rearrange("t s -> s t"))
                    nc.vector.tensor_tensor(
                        sc[:, off:off + 512], sc[:, off:off + 512],
                        mdt, mybir.AluOpType.add)
            ex = EX.tile([128, 1024], BF16, tag="ex", name="ex")
            exs[(h, jj, i)] = ex
            lo = 512 * (min(jbs) - 2 * jj) + block_trim(i, min(jbs))
            hi = 512 * (max(jbs) - 2 * jj) + 512
            escale = (SCALE if has_gen else scl_sb[:, i, h:h + 1])
            nc.scalar.activation(
                ex[:, lo:hi], sc[:, lo:hi],
                mybir.ActivationFunctionType.Exp, scale=escale)
            for jb in jbs:
                if cls[i][jb] != "tri":
                    continue
                # zero the above-diagonal half of the 128-wide strip
                # (gpsimd; consumed by the AV matmuls a block later)
                off = 512 * (jb - 2 * jj)
                strip = off + 128 * (i - 4 * jb)
                nc.gpsimd.tensor_tensor(
                    ex[:, strip:strip + 128],
                    ex[:, strip:strip + 128],
                    trikeep_sb, mybir.AluOpType.mult)

        def emit_av_iter(h, jj, jb, i):
            if (h, jj) not in avs:
                avs[(h, jj)] = PS.tile([128, 1024], F32, tag="av", name="av")
            av = avs[(h, jj)]
            ex = exs[(h, jj, i)]
            off = 512 * (jb - 2 * jj)
            trim = block_trim(i, jb)
            nc.tensor.matmul(
                av[:, off + trim:off + 512],
                v_aug[:, i, h, :],
                ex[:, off + trim:off + 512],
                start=(first_i[jb] == i),
                stop=(last_i[jb] == i))

        def emit_block_tail(h, jj):
            """den extraction + reciprocal + broadcast + y normalize."""
            par, pair = h % 2, h // 2
            rlo, rhi = 64 * par, 64 * par + 64
            dlo = 64 - rlo
            av = avs.pop((h, jj))
            for i in ivals_of(h, jj):
                exs.pop((h, jj, i), None)
            for jb in (2 * jj, 2 * jj + 1):
                if first_i[jb] is None:
                    off = 512 * (jb - 2 * jj)
                    nc.vector.memset(av[:, off:off + 512], 1.0)
            den_sb = NR.tile([1, 1024], F32, tag="den_sb", name="den_sb")
            nc.vector.tensor_scalar_mul(den_sb, av[dlo:dlo + 1, :], 1.0)
            den_pk = NR.tile([128, 8], F32, tag="den_pk", name="den_pk")
            nc.scalar.dma_start(out=den_pk, in_=den_sb)
            rec_pk = NR.tile([128, 8], F32, tag="rec_pk", name="rec_pk")
            nc.vector.reciprocal(rec_pk, den_pk)
            rec_bf = NR.tile([128, 8], BF16, tag="rec_bf", name="rec_bf")
            nc.vector.tensor_copy(out=rec_bf, in_=rec_pk)
            hsl2 = slice(1024 * jj, 1024 * (jj + 1))
            nc.scalar.dma_start(out=rec_dram[h, hsl2], in_=rec_bf)
            rrow_ap = rec_dram[h, hsl2]
            rec_bc = bass.AP(
                tensor=rrow_ap.tensor,
                offset=rrow_ap.offset,
                ap=[[0, 64]] + [list(p) for p in rrow_ap.ap])
            rec = NR.tile([64, 1024], BF16, tag="rec", name="rec")
            nc.scalar.dma_start(out=rec, in_=rec_bc)
            nc.vector.tensor_tensor(
                yT[rlo:rhi, pair, hsl2],
                av[rlo:rhi, :], rec,
                mybir.AluOpType.mult)

        def run_block_pair(blk, prev, fillers, cadence):
            """Interleave scores/exp of `blk` with AVs of `prev`."""
            sc_ops = ([lambda h=blk[0], jj=blk[1], i=i: emit_sc_iter(h, jj, i)
                       for i in ivals_of(*blk)] if blk else [])
            av_ops = []
            if prev:
                hP, jjP = prev
                for jb in (2 * jjP, 2 * jjP + 1):
                    for i in range(NT):
                        if cls[i][jb] != "skip":
                            av_ops.append(
                                lambda h=hP, jj=jjP, jb=jb, i=i:
                                emit_av_iter(h, jj, jb, i))
            seq = ([(((k + 0.5) / len(sc_ops)), 0, f)
                    for k, f in enumerate(sc_ops)]
                   + [(((k + 0.5) / len(av_ops)), 1, f)
                      for k, f in enumerate(av_ops)])
            seq.sort(key=lambda t: (t[0], t[1]))
            for n, (_, _, f) in enumerate(seq):
                f()
                if fillers and n % cadence == cadence - 1:
                    fillers.pop(0)()
            if prev:
                emit_block_tail(*prev)

        # ================= stage C =================
        def emit_po(ec, tb):
            po = PS.tile([128, 1024], F32, tag="sc", name="po")
            for dc in range(2):
                nc.tensor.matmul(
                    po[:, 0:512],
                    wo_sb[:, dc, 128 * ec:128 * (ec + 1)],
                    yT[:, dc, 512 * tb:512 * (tb + 1)],
                    start=(dc == 0), stop=(dc == 1))
            os = OS.tile([128, 512], BF16, tag="os", name="os")
            nc.vector.tensor_copy(out=os, in_=po[:, 0:512])
            nc.sync.dma_start(
                out=outP[128 * ec:128 * (ec + 1), 512 * tb:512 * (tb + 1)],
                in_=os)

        # ================= emission =================
        for tt in range(12):
            emit_tile(tt)

        fillers = [lambda tt=tt: emit_tile(tt) for tt in range(12, 16)]
        fillers.append(flush_transposes)
        blocks = [(h, 0) for h in range(HPG)] + [(h, 1) for h in range(HPG)]
        po_started = [False]

        def add_po_fillers():
            if not po_started[0]:
                po_started[0] = True
                for ec in range(8):
                    for tb in (0, 1):
                        fillers.append(
                            lambda ec=ec, tb=tb: emit_po(ec, tb))

        prev = None
        for bi, blk in enumerate(blocks):
            if bi == HPG:
                flush_transposes()
            if bi == HPG + 1:
                # all jj0 tails are emitted once prev==(h3,0) is retired
                add_po_fillers()
            run_block_pair(blk, prev, fillers, 3 if bi < 2 else 5)
            prev = blk
        add_po_fillers()
        run_block_pair(None, prev, fillers, 3)

        while fillers:
            fillers.pop(0)()
        for ec in range(8):
            for tb in (2, 3):
                emit_po(ec, tb)
    _split_excess_waits(nc)
    return nc


def kernel(x, ve, sa_lambdas, attn_mask, qkvo_w):
    global LAST_EXEC_NS
    x = np.ascontiguousarray(np.asarray(x, np.float32))
    ve = np.ascontiguousarray(np.asarray(ve, np.float32))
    sa_lambdas = np.asarray(sa_lambdas, np.float32)
    attn_mask = np.asarray(attn_mask, np.float32)
    qkvo_w = np.asarray(qkvo_w, np.float32)

    ropeC, ropeS = _rope_tables()
    mask = attn_mask[0, 0]
    cls = _classify_blocks(mask)
    for jb in range(NJ):
        valid = [i for i in range(NT) if cls[i][jb] != "skip"]
        if valid and cls[valid[0]][jb] == "tri" and valid[0] - 4 * jb > 0:
            # the tri fast path assumes the strip starts inside the block
            cls[valid[0]][jb] = "gen"
    has_gen = any(c == "gen" for row in cls for c in row)

    nc = _build_program(cls, has_gen)

    part = np.arange(128)
    trikeep = (part[:, None] <= part[None, :]).astype(np.float32)      # [p, c]
    trikeep = np.ascontiguousarray(trikeep).astype(ml_dtypes.bfloat16)
    lam0 = np.full((128, 1), sa_lambdas[0], np.float32)
    rcP = np.ascontiguousarray(
        ropeC.reshape(NT, 128, 16).transpose(1, 0, 2)).astype(ml_dtypes.bfloat16)
    rsP = np.ascontiguousarray(
        ropeS.reshape(NT, 128, 16).transpose(1, 0, 2)).astype(ml_dtypes.bfloat16)
    maskdiv = (mask / SCALE).astype(np.float32) if has_gen else None

    in_maps = []
    for c in range(8):
        b, g = c // G, c % G
        sl = slice(GD * g, GD * (g + 1))
        wqkvT = np.concatenate([qkvo_w[k][sl, :] for k in range(3)], 0).T
        # xP[p, tt, ds, c] = x[b][128*tt + c, 128*ds + p]
        xco = np.ascontiguousarray(
            x[b].T.reshape(8, 128, NT, 128).transpose(1, 2, 0, 3)
        ).astype(ml_dtypes.bfloat16)
        m = {
            "xP": xco,
            "wqP": np.ascontiguousarray(
                wqkvT.reshape(8, 128, 3 * GD).transpose(1, 0, 2)
            ).astype(ml_dtypes.bfloat16),
            "woP": np.ascontiguousarray(
                qkvo_w[3][:, sl].T.reshape(2, 128, DIM).transpose(1, 0, 2)
            ).astype(ml_dtypes.bfloat16),
            "veP": np.ascontiguousarray(
                (ve[b][:, sl] * sa_lambdas[1]).reshape(NT, 128, GD)
                .transpose(1, 0, 2)
            ).astype(ml_dtypes.bfloat16),
            "lamP": lam0,
            "rcP": rcP,
            "rsP": rsP,
            "triP": trikeep,
        }
        if has_gen:
            m["maskdiv"] = maskdiv
        in_maps.append(m)

    res = run_bass_kernel_spmd(nc, in_maps, core_ids=list(range(8)),
                               trace=TRACE)
    if TRACE:
        LAST_EXEC_NS = res.exec_time_ns

    out = np.zeros((B, T, DIM), np.float32)
    for c in range(8):
        out[c // G] += res.results[c]["outP"].astype(np.float32).T
    return out
